# revision 1
# baseline (speedup 1.0000x reference)
"""Trainium2 Bass kernel for conv3d(8->16, k3, valid) + /2 + maxpool2 + global-mean
+ bias + channel-sum, batch 128 sharded over 8 NeuronCores.

Strategy: the PE array is addressed as 16 concurrent 32x32 tiles
(tile_position).  Each tile computes a [K=32=(h_in 4, i 8)] x [M=32=(h_out 2,
o 16)] matmul whose stationary operand is a Toeplitz expansion of the conv
weights over the H dimension (75% dense).  Row groups = 4 h-window phases,
col groups = 4 batch elements.  The 27-tap kernel reduces to 9
PSUM-accumulated matmuls per tile, sweeping (kd, kw) via shifted reads of the
flattened (d, w) free dim.  Inputs are pre-transposed on the host to
[b, h, i, d, w] so every DMA is contiguous; bf16 in, fp32 PSUM out.
MaxPool w/d pairs run on DVE (pool_max); the h-pair max, global mean, bias
and channel sum fold into host-side numpy on the tiny [128, 105] partials.
"""

import numpy as np
import ml_dtypes

import concourse.bass as bass
import concourse.tile as tile
from concourse import mybir
from concourse.bass_utils import run_bass_kernel_spmd
import bass_rust

# ---------------------------------------------------------------------------
# Fixup: this walrus build only accepts a single sem-wait per instruction.
# After the program is fully built, hoist extra waits onto same-engine NoOps
# inserted immediately before the offending instruction (same queue => the
# waits still gate it).
# ---------------------------------------------------------------------------
def _split_multiwaits(nc):
    cur_list = nc.cur_bb.bb.instructions

    def make_nop(engine):
        bi = nc.engines[engine].nop(nofuse=True, hint="waitsplit")
        assert cur_list[-1] is bi.ins
        cur_list.pop()
        return bi.ins

    for f in nc.m.functions:
        for bb in f.blocks:
            il = bb.instructions
            if not any(
                i.sync_info is not None and len(i.sync_info.on_wait) > 1 for i in il
            ):
                continue
            new = []
            for inst in il:
                si = inst.sync_info
                waits = list(si.on_wait) if si is not None else []
                if len(waits) > 1:
                    for w in waits[:-1]:
                        nop = make_nop(inst.engine)
                        nop.sync_info = bass_rust.SyncInfo(on_wait=[w], on_update=[])
                        new.append(nop)
                    si.on_wait = [waits[-1]]
                new.append(inst)
            bb.instructions[:] = new

N_CORES = 8
B_PER_CORE = 16
BF16 = mybir.dt.bfloat16
F32 = mybir.dt.float32
NPBF16 = ml_dtypes.bfloat16

# h-window tensors: (dram row offset, partitions, row groups)
#   t=0: A0 = h 0..15   -> hb 0,2,4,6
#   t=1: A1 = h 16..31  -> hb 8,10,12,14
#   t=2: B0 = h 2..17   -> hb 1,3,5,7
#   t=3: B1 = h 18..29  -> hb 9,11,13
T_PARTS = [128, 128, 128, 96]
T_RGS = [4, 4, 4, 3]
FREE = 520  # (d=16, w=32) flattened = 512, +8 zero pad for shifted reads


def _build_program(repeat=1, variant="full"):
    nc = bass.Bass("TRN2", target_bir_lowering=False, debug=False, num_devices=1)
    X = nc.declare_dram_parameter("X", [B_PER_CORE, 480, FREE], BF16, isOutput=False)
    W = nc.declare_dram_parameter("W", [128, 288], BF16, isOutput=False)
    OUT = nc.declare_dram_parameter("OUT", [16, 4, 128, 105], F32, isOutput=True)

    with tile.TileContext(nc) as tc:
        with (
            tc.tile_pool(name="wpool", bufs=1) as wpool,
            tc.tile_pool(name="xpool", bufs=1) as xpool,
            tc.tile_pool(name="psum", bufs=2, space="PSUM") as psum,
            tc.tile_pool(name="s1pool", bufs=4) as s1pool,
            tc.tile_pool(name="s2pool", bufs=4) as s2pool,
        ):
            import contextlib
            loop_cm = tc.For_i(0, repeat, 1) if repeat > 1 else contextlib.nullcontext()
            with loop_cm:
                w_sb = wpool.tile([128, 288], BF16)
                nc.sync.dma_start(w_sb[:], W[:])

                xt = {}
                for b in range(B_PER_CORE):
                    row = 0
                    for t in range(4):
                        p = T_PARTS[t]
                        xtile = xpool.tile([128, FREE], BF16, name=f"x_{b}_{t}", tag=f"x_{b}_{t}")
                        nc.sync.dma_start(xtile[:p, :], X[b, row : row + p, :])
                        xt[b, t] = xtile
                        row += p

                for g in range(4):
                    for t in range(4):
                        n_rg = T_RGS[t]
                        ps = [psum.tile([128, 448], F32, name=f"ps_{g}_{t}_{r}", tag=f"ps{r}") for r in range(n_rg)]
                        nmm = 0 if variant == "nomm" else 448 if variant != "smalln" else 64
                        for s in range(9):
                            kd, kw = s // 3, s % 3
                            off = kd * 32 + kw
                            for r in range(n_rg):
                                for j in range(4):
                                    if nmm == 0:
                                        continue
                                    nc.tensor.matmul(
                                        out=ps[r][32 * j : 32 * j + 32, :nmm],
                                        lhsT=w_sb[32 * r : 32 * r + 32, 32 * s : 32 * s + 32],
                                        rhs=xt[4 * g + j, t][32 * r : 32 * r + 32, off : off + nmm],
                                        start=(s == 0),
                                        stop=(s == 8),
                                        tile_position=(32 * r, 32 * j),
                                        skip_group_check=True,
                                    )
                        for r in range(n_rg if variant != "nopool" else 0):
                            # w-pair max: psum (d 14, w 32) -> s1 (d 14, wp 15).
                            # TT can't read two PSUM operands; ACT stages the odd
                            # columns into SBUF first.
                            pv = ps[r][:].rearrange("p (d wp c) -> p d wp c", wp=16, c=2)
                            podd = s1pool.tile([128, 210], F32, tag="podd")
                            nc.scalar.copy(podd[:], pv[:, :, :15, 1])
                            s1 = s1pool.tile([128, 210], F32)
                            nc.vector.tensor_max(s1[:], pv[:, :, :15, 0], podd[:])
                            # d-pair max: s1 (d 14, wp 15) -> s2 (dp 7, wp 15)
                            s2 = s2pool.tile([128, 105], F32)
                            sv = s1[:].rearrange("p (dp c wp) -> p dp c wp", c=2, wp=15)
                            nc.vector.tensor_max(s2[:], sv[:, :, 0, :], sv[:, :, 1, :])
                            nc.sync.dma_start(OUT[4 * g + t, r, :, :], s2[:])
    _split_multiwaits(nc)
    return nc


def _build_program_v2(repeat=1, col_tiling=True, variant="full"):
    """Space-to-depth scheme: host packs X2[b] = [(hi,wi,i)=128, (d,hb,wb)=3600]
    so the conv is a K=128 dense-column matmul, M=64=(hl,wl,o), 3 PSUM-accum
    steps over kd via free-dim shifts.  Two batch elements run concurrently in
    the two 64-column halves of the PE array (col tiling).  Chunk = one output
    d-pair: N=450.  Device reduces the d-pair max; h/w-pair maxes + sums happen
    on the host over the small [128,225] partials."""
    nc = bass.Bass("TRN2", target_bir_lowering=False, debug=False, num_devices=1)
    X = nc.declare_dram_parameter("X", [B_PER_CORE, 128, 3600], BF16, isOutput=False)
    W = nc.declare_dram_parameter("W", [128, 384], BF16, isOutput=False)
    OUT = nc.declare_dram_parameter("OUT", [8, 7, 128, 225], BF16, isOutput=True)

    with tile.TileContext(nc) as tc:
        with (
            tc.tile_pool(name="wpool", bufs=1) as wpool,
            tc.tile_pool(name="xpool", bufs=1) as xpool,
            tc.tile_pool(name="psum", bufs=6, space="PSUM") as psum,
            tc.tile_pool(name="mpool", bufs=4) as mpool,
        ):
            import contextlib
            loop_cm = tc.For_i(0, repeat, 1) if repeat > 1 else contextlib.nullcontext()
            with loop_cm:
                w_sb = wpool.tile([128, 384], BF16)
                nc.sync.dma_start(w_sb[:], W[:])
                xt = {}
                for b in range(B_PER_CORE):
                    xtile = xpool.tile([128, 3600], BF16, name=f"x2_{b}", tag=f"x2_{b}")
                    nc.sync.dma_start(xtile[:], X[b, :, :])
                    xt[b] = xtile

                for pair in range(8 if variant != "dmaonly" else 0):
                    b0, b1 = 2 * pair, 2 * pair + 1
                    for c in range(7):
                        pt = psum.tile([128, 450], F32, name=f"ps_{pair}_{c}", tag="ps")
                        for kd in range(3):
                            off = (2 * c + kd) * 225
                            if col_tiling and variant != "nocol":
                                nc.tensor.matmul(
                                    out=pt[0:64, :],
                                    lhsT=w_sb[:, 128 * kd : 128 * kd + 64],
                                    rhs=xt[b0][:, off : off + 450],
                                    start=(kd == 0), stop=(kd == 2),
                                    tile_position=(0, 0), skip_group_check=True,
                                )
                                nc.tensor.matmul(
                                    out=pt[64:128, :],
                                    lhsT=w_sb[:, 128 * kd + 64 : 128 * kd + 128],
                                    rhs=xt[b1][:, off : off + 450],
                                    start=(kd == 0), stop=(kd == 2),
                                    tile_position=(0, 64), skip_group_check=True,
                                )
                            else:
                                nc.tensor.matmul(
                                    out=pt[0:64, :],
                                    lhsT=w_sb[:, 128 * kd : 128 * kd + 64],
                                    rhs=xt[b0][:, off : off + 450],
                                    start=(kd == 0), stop=(kd == 2),
                                    skip_group_check=True,
                                )
                        if variant == "nopool":
                            continue
                        # d-pair max: cols 0:225 (d even) vs 225:450 (d odd)
                        podd = mpool.tile([128, 225], F32, tag="podd")
                        nc.scalar.copy(podd[:], pt[:, 225:450])
                        m1 = mpool.tile([128, 225], BF16, tag="m1")
                        nc.vector.tensor_max(m1[:], pt[:, 0:225], podd[:])
                        nc.sync.dma_start(OUT[pair, c, :, :], m1[:])
    _split_multiwaits(nc)
    return nc


def _host_inputs_v2(x, conv_weight):
    """x: [128,8,16,32,32] f32 -> per-core {X, W} with space-to-depth layout."""
    w_half = (conv_weight.astype(np.float64) / 2.0).astype(np.float32)

    # W: [128, 3*128]; per kd a [128, 64] block = W2[(hi,wi,i), (hl,wl,o)],
    # duplicated in cols 64:128 for the second col tile.
    W_np = np.zeros((128, 384), dtype=np.float32)
    for kd in range(3):
        blk = np.zeros((4, 4, 8, 2, 2, 16), dtype=np.float32)  # hi wi i hl wl o
        for hi in range(4):
            for hl in range(2):
                kh = hi - hl
                if not 0 <= kh <= 2:
                    continue
                for wi in range(4):
                    for wl in range(2):
                        kw = wi - wl
                        if not 0 <= kw <= 2:
                            continue
                        # [o, i] -> [i, o]
                        blk[hi, wi, :, hl, wl, :] = w_half[:, :, kd, kh, kw].T
        b2 = blk.reshape(128, 64)
        W_np[:, 128 * kd : 128 * kd + 64] = b2
        W_np[:, 128 * kd + 64 : 128 * kd + 128] = b2
    W_np = W_np.astype(NPBF16)

    # X2[b] = [(hi,wi,i)=128, (d=16, hb=15, wb=15)]
    #       = x[b, i, d, 2hb+hi, 2wb+wi]
    B = x.shape[0]
    X2 = np.empty((B, 4, 4, 8, 16, 15, 15), dtype=np.float32)
    for hi in range(4):
        for wi in range(4):
            X2[:, hi, wi] = x[:, :, :, hi : hi + 30 : 2, wi : wi + 30 : 2]
    X2 = X2.reshape(B, 128, 3600).astype(NPBF16)

    in_maps = []
    for c in range(N_CORES):
        in_maps.append({"X": X2[c * B_PER_CORE : (c + 1) * B_PER_CORE], "W": W_np})
    return in_maps


def _host_finish_v2(results, conv_bias, bias):
    C = float(
        np.sum(conv_bias.astype(np.float64) / 2.0 + bias.reshape(-1).astype(np.float64))
    )
    out = np.zeros((128,), dtype=np.float64)
    for c in range(N_CORES):
        O = results[c]["OUT"].astype(np.float32)  # [8, 7, 128, 225]
        # partitions: (b_in_pair 2, hl 2, wl 2, o 16)
        O = O.reshape(8, 7, 2, 2, 2, 16, 225)
        m = O.max(axis=(3, 4))  # max over hl, wl -> [8, 7, 2, 16, 225]
        s = m.sum(axis=(1, 3, 4), dtype=np.float64)  # [8, 2]
        out[c * B_PER_CORE : (c + 1) * B_PER_CORE] = s.reshape(16)
    out = out / 1575.0 + C
    return out.astype(np.float32).reshape(128, 1, 1, 1)


_PROGRAM = None


def _get_program():
    global _PROGRAM
    if _PROGRAM is None:
        _PROGRAM = _build_program_v2()
    return _PROGRAM


def _host_inputs(x, conv_weight, conv_bias, bias):
    """Build per-core input maps. x: [128,8,16,32,32] f32."""
    w_half = (conv_weight.astype(np.float64) / 2.0).astype(np.float32)

    # Weights: [128, 288] = 4 identical row bands x 9 steps of [32, 32].
    W_np = np.zeros((128, 288), dtype=np.float32)
    for s in range(9):
        kd, kw = s // 3, s % 3
        blk = np.zeros((32, 32), dtype=np.float32)
        for hi in range(4):
            for hl in range(2):
                kh = hi - hl
                if 0 <= kh <= 2:
                    # blk[hi*8 + i, hl*16 + o] = w_half[o, i, kd, kh, kw]
                    blk[hi * 8 : hi * 8 + 8, hl * 16 : hl * 16 + 16] = w_half[
                        :, :, kd, kh, kw
                    ].T
        for r in range(4):
            W_np[32 * r : 32 * r + 32, 32 * s : 32 * s + 32] = blk
    W_np = W_np.astype(NPBF16)

    # x -> [b, h, i, d, w] then stack h-window copies.
    xt = np.ascontiguousarray(x.transpose(0, 3, 1, 2, 4))  # [128, 32, 8, 16, 32]
    xt = xt.reshape(128, 32, 8, 512)
    in_maps = []
    for c in range(N_CORES):
        xs = xt[c * B_PER_CORE : (c + 1) * B_PER_CORE]  # [16, 32, 8, 512]
        X_np = np.zeros((B_PER_CORE, 480, FREE), dtype=np.float32)
        A = xs.reshape(B_PER_CORE, 256, 512)  # h 0..31
        Bc = xs[:, 2:30].reshape(B_PER_CORE, 224, 512)  # h 2..29
        X_np[:, 0:256, :512] = A
        X_np[:, 256:480, :512] = Bc
        in_maps.append({"X": X_np.astype(NPBF16), "W": W_np})
    return in_maps


def _host_finish(results, conv_bias, bias):
    """Combine per-core OUT tensors into the final [128,1,1,1] output."""
    C = float(np.sum(conv_bias.astype(np.float64) / 2.0 + bias.reshape(-1).astype(np.float64)))
    out = np.zeros((128,), dtype=np.float64)
    for c in range(N_CORES):
        O = results[c]["OUT"].astype(np.float64)  # [16, 4, 128, 105]
        for g in range(4):
            for t in range(4):
                for r in range(T_RGS[t]):
                    data = O[4 * g + t, r]  # [128, 105]
                    for j in range(4):
                        blk = data[32 * j : 32 * j + 32]  # [(hl 2, o 16), 105]
                        m = np.maximum(blk[:16], blk[16:32])  # [16, 105]
                        out[c * B_PER_CORE + 4 * g + j] += m.sum()
    out = out / 1575.0 + C
    return out.astype(np.float32).reshape(128, 1, 1, 1)


def _run(x, conv_weight, conv_bias, bias, trace=False):
    nc = _get_program()
    in_maps = _host_inputs_v2(
        np.asarray(x, dtype=np.float32),
        np.asarray(conv_weight, dtype=np.float32),
    )
    res = run_bass_kernel_spmd(
        nc, in_maps, core_ids=list(range(N_CORES)), trace=trace
    )
    out = _host_finish_v2(
        res.results,
        np.asarray(conv_bias, dtype=np.float32),
        np.asarray(bias, dtype=np.float32),
    )
    return out, res


def kernel(x, conv_weight, conv_bias, bias):
    out, _ = _run(x, conv_weight, conv_bias, bias, trace=False)
    return out


def kernel_traced(x, conv_weight, conv_bias, bias):
    """For test.py: returns (output, BassKernelResults with exec_time_ns)."""
    return _run(x, conv_weight, conv_bias, bias, trace=True)

